# revision 14
# baseline (speedup 1.0000x reference)
"""CantorAttention Trainium2 kernel — banded block-sparse edition.

Problem (hardcoded): B=2, S=2048, DIM=512, H=8 heads, D=64, K=64 routes.
  qkv = x @ w_qkv + b_qkv ; per-head sparse attention over routes[q, :] ;
  out = attn_out @ w_out + b_out.

Strategy (8 cores): shard batch x head-pairs. Core i handles batch i//4 and
heads (2*(i%4), 2*(i%4)+1). Routes are shared across batch/heads.

The sparse attention is made BANDED by a host-computed permutation rho of
the sequence (barycenter seriation of the bipartite route graph — for the
Cantor routes this recovers coordinate order and every query's 64 routes
land in a <=128-wide window of permuted key space). Queries and keys are
both processed in rho order; only the nonzero 128x128 (key-tile x
query-tile) blocks of the routed score matrix are computed:

  P[k, q]  = C~[k, q] * exp(scale * (k_vec . k_vec))   (count mask, 0 off-route)
  AV_h     = [V_h | 1]^T @ P      -> rows 0..63 out, row 64 = denominator
  out_h    = AV_h / denom + bv_h  ;  partial = concat_h(out_h)^T @ w_out
Host gathers: final[b][perm] += partial_core ; final += b_out.

Exact softmax over the 64 routed scores for ANY routes input (block list
is derived from the actual routes; a bad permutation only costs speed).
All matmuls bf16 with fp32 PSUM accumulation; exp on ScalarE; mask-multiply
on VectorE; denominators broadcast across partitions via K=1 matmuls of
sel rows (no zeroed scratch needed); V is produced directly in [key, d]
layout (x^T blocks as stationary operand — no on-chip transposes);
input DMAs split across both HWDGE rings (sync + scalar) in critical-path
order; fp16 output stores.
"""

import numpy as np
import ml_dtypes

import concourse.bacc as bacc
import concourse.mybir as mybir
import concourse.tile as tile
from concourse.bass_utils import run_bass_kernel_spmd

BF16 = mybir.dt.bfloat16
F32 = mybir.dt.float32
FP16 = mybir.dt.float16
NPBF16 = ml_dtypes.bfloat16

B = 2
S = 2048
DIM = 512
H = 8
D = 64
KR = 64
SCALE = 0.125

P = 128
NT = S // P       # 16 tiles of 128 (queries and keys)
QC = 512          # query group width (psum bank)
NQG = S // QC     # 4 query groups
NC4 = DIM // P    # 4 contraction chunks
SGB = 4           # max blocks per (qt, h) score tile ([128, 512] psum)

_CACHE = {}


# ----------------------------------------------------------------------
# Host-side planning: permutation + block structure from routes alone.

def _block_cost(pos, routes):
    qt = pos[: routes.shape[0]] // P
    rp = pos[routes] // P
    return len(np.unique(qt[:, None] * NT + rp))


def _seriate(routes):
    """Find perm (rho-position -> original index) making the route matrix
    banded. Barycenter sweeps; keeps the best block count seen."""
    routes = np.asarray(routes, np.int64)
    n, k = routes.shape
    qidx = np.repeat(np.arange(n), k)
    kidx = routes.ravel()
    best_perm = np.arange(n)
    best_cost = _block_cost(np.arange(n), routes)
    for variant in ("q", "qk"):
        pos = np.arange(n, dtype=np.float64)
        stale = 0
        for _ in range(48):
            bq = pos[routes].mean(1)
            if variant == "qk":
                sums = np.zeros(n)
                cnts = np.zeros(n)
                np.add.at(sums, kidx, pos[qidx])
                np.add.at(cnts, kidx, 1)
                bk = np.where(cnts > 0, sums / np.maximum(cnts, 1.0), pos)
                b = 0.5 * (bq + bk)
            else:
                b = bq
            order = np.argsort(b, kind="stable")
            npos = np.empty(n)
            npos[order] = np.arange(n)
            pos = npos
            c = _block_cost(pos.astype(np.int64), routes)
            if c < best_cost:
                best_cost, best_perm, stale = c, order.copy(), 0
            else:
                stale += 1
                if stale >= 6:
                    break
    return best_perm, best_cost


def plan_from_routes(routes):
    """-> (perm, plan, nblocks); plan[qt] = tuple of key tiles needed."""
    routes = np.asarray(routes, np.int64)
    perm, _ = _seriate(routes)
    pos = np.empty(S, np.int64)
    pos[perm] = np.arange(S)
    plan = []
    for qt in range(NT):
        qs = perm[qt * P:(qt + 1) * P]
        kts = np.unique(pos[routes[qs]] // P)
        plan.append(tuple(int(x) for x in kts))
    nblocks = sum(len(x) for x in plan)
    return perm, tuple(plan), nblocks


def _chunk4(seq):
    return [seq[i:i + 4] for i in range(0, len(seq), 4)]


# ----------------------------------------------------------------------
# Device program.

def build_nc(plan, nblocks):
    nc = bacc.Bacc(
        "TRN2",
        target_bir_lowering=False,
        debug=False,
        num_devices=8,
    )

    xt_d = nc.dram_tensor("xt", [P, NC4 * S], BF16, kind="ExternalInput").ap()
    wq_d = nc.dram_tensor("wq", [P, NC4 * P], BF16, kind="ExternalInput").ap()
    wk_d = nc.dram_tensor("wk", [P, NC4 * P], BF16, kind="ExternalInput").ap()
    wv_d = nc.dram_tensor("wv", [P, NC4 * P], BF16, kind="ExternalInput").ap()
    # rows 0/1: q/k bias as [1, 128] rows (for the K=1 bias matmuls)
    bqk_d = nc.dram_tensor("bqk", [2, P], BF16, kind="ExternalInput").ap()
    bv_d = nc.dram_tensor("bv", [P, 1], F32, kind="ExternalInput").ap()
    ct_d = nc.dram_tensor("ct", [P, nblocks * P], BF16, kind="ExternalInput").ap()
    wo_d = nc.dram_tensor("wo", [P, DIM], BF16, kind="ExternalInput").ap()
    out_d = nc.dram_tensor("out", [S, DIM], FP16, kind="ExternalOutput").ap()

    # enum offset of first block of each query tile
    boff = np.cumsum([0] + [len(x) for x in plan])

    with tile.TileContext(nc) as tc:
        with tc.tile_pool(name="persist", bufs=1) as pp:
            # --- input DMAs, split across both HWDGE rings -------------
            xt_big = pp.tile([P, NC4 * S], BF16, tag="xtb", name="xt_big")
            w_sb = {}
            wts = {}
            for name, wd in (("q", wq_d), ("k", wk_d), ("v", wv_d)):
                wt = pp.tile([P, NC4 * P], BF16, tag=f"w{name}b", name=f"w{name}_big")
                wts[name] = wt
                for c in range(NC4):
                    w_sb[(name, c)] = wt[:, c * P:(c + 1) * P]
            bqk_sb = {}
            for bi in range(2):
                bqk_sb[bi] = pp.tile([P, P], BF16, tag=f"bqk{bi}", name=f"bqk{bi}_sb")
            bv_sb = pp.tile([P, 1], F32, tag="bv", name="bv_sb")
            ct_big = pp.tile([P, nblocks * P], BF16, tag="ctb", name="ct_big")
            wo_sb = pp.tile([P, DIM], BF16, tag="wo")

            nc.sync.dma_start(out=xt_big[:, 0:1024], in_=xt_d[:, 0:1024])
            nc.scalar.dma_start(out=wts["q"][:], in_=wq_d[:, :])
            nc.scalar.dma_start(out=wts["k"][:], in_=wk_d[:, :])
            nc.scalar.dma_start(out=bqk_sb[0][0:1, :], in_=bqk_d[0:1, :])
            nc.scalar.dma_start(out=bqk_sb[1][0:1, :], in_=bqk_d[1:2, :])
            nc.sync.dma_start(out=xt_big[:, 1024:2048], in_=xt_d[:, 1024:2048])
            nc.scalar.dma_start(out=wts["v"][:], in_=wv_d[:, :])
            lo, hi = 0, boff[4] * P
            nc.scalar.dma_start(out=ct_big[:, lo:hi], in_=ct_d[:, lo:hi])
            nc.sync.dma_start(out=xt_big[:, 2048:4096], in_=xt_d[:, 2048:4096])
            nc.sync.dma_start(out=xt_big[:, 4096:6144], in_=xt_d[:, 4096:6144])
            nc.sync.dma_start(out=xt_big[:, 6144:8192], in_=xt_d[:, 6144:8192])
            nc.scalar.dma_start(out=bv_sb[:], in_=bv_d[:, :])
            nc.scalar.dma_start(out=wo_sb[:], in_=wo_d[:, :])
            for g in range(1, NQG):
                lo, hi = boff[4 * g] * P, boff[4 * (g + 1)] * P
                nc.scalar.dma_start(out=ct_big[:, lo:hi], in_=ct_d[:, lo:hi])

            # --- small constants ---------------------------------------
            ones_sb = pp.tile([P, QC], BF16, tag="ones", name="ones_sb")
            nc.gpsimd.memset(ones_sb[0:1, :], 1.0)
            # sel rows for the K=1 denominator-broadcast matmuls: only row
            # 0 is ever read, so no full-tensor zeroing is needed anywhere.
            sel_sb = {}
            for h in range(2):
                t = pp.tile([P, P], BF16, tag=f"sel{h}", name=f"sel{h}")
                nc.gpsimd.memset(t[0:1, :], 0.0)
                nc.gpsimd.memset(t[0:1, h * D:(h + 1) * D], 1.0)
                sel_sb[h] = t

            # q^T/k^T per-head, rows 64-127 zero-padded (K=128 stationary
            # operands keep FWL enabled)
            qkvt = {}
            for ei, (name, h) in enumerate(
                    ((n, hh) for n in ("q", "k") for hh in range(2))):
                t = pp.tile([P, S], BF16, tag=f"{name}t{h}", name=f"{name}t{h}")
                nc.gpsimd.memset(t[D:P, :], 0.0)
                qkvt[(name, h)] = t
            # V tiles in [key, d] layout with a ones column at col 64
            v_sb = {}
            for h in range(2):
                for kt in range(NT):
                    t = pp.tile([P, D + 1], BF16, tag=f"v{h}_{kt}", name=f"v{h}_{kt}")
                    nc.gpsimd.memset(t[:, D:D + 1], 1.0)
                    v_sb[(h, kt)] = t

            den_sb = {}
            for h in range(2):
                den_sb[h] = pp.tile([P, S], BF16, tag=f"den{h}", name=f"den{h}")
            on_sb = pp.tile([P, S], BF16, tag="on")

            # Phase 1: per query group, K^T/Q^T (+bias) and V tiles.
            with tc.tile_pool(name="psum_pre", bufs=2, space="PSUM") as pre, \
                 tc.tile_pool(name="psum_v", bufs=4, space="PSUM") as vpl:
                for qc in range(NQG):
                    xq = xt_big[:, qc * 2048:(qc + 1) * 2048]
                    for bi, name in ((1, "k"), (0, "q")):
                        ps = pre.tile([P, QC], F32, tag="qkps", name="qkps")
                        for c in range(NC4):
                            nc.tensor.matmul(
                                ps[:],
                                lhsT=w_sb[(name, c)],
                                rhs=xq[:, c * QC:(c + 1) * QC],
                                start=(c == 0),
                                stop=False,
                            )
                        nc.tensor.matmul(
                            ps[:],
                            lhsT=bqk_sb[bi][0:1, :],
                            rhs=ones_sb[0:1, :],
                            start=False,
                            stop=True,
                        )
                        for h in range(2):
                            hd = h * D
                            nc.vector.tensor_copy(
                                out=qkvt[(name, h)][0:D, qc * QC:(qc + 1) * QC],
                                in_=ps[hd:hd + D, :],
                            )
                    for kk in range(4):
                        kt = qc * 4 + kk
                        vp = vpl.tile([P, P], F32, tag="vps", name="vps")
                        for c in range(NC4):
                            nc.tensor.matmul(
                                vp[:],
                                lhsT=xq[:, c * QC + kk * P: c * QC + (kk + 1) * P],
                                rhs=w_sb[("v", c)],
                                start=(c == 0),
                                stop=(c == NC4 - 1),
                            )
                        nc.vector.tensor_copy(out=v_sb[(0, kt)][:, 0:D], in_=vp[:, 0:D])
                        nc.vector.tensor_copy(out=v_sb[(1, kt)][:, 0:D], in_=vp[:, D:P])

            # Phase 2 + 3: banded masked attention, then normalize+project,
            # pipelined per 512-query group.
            with tc.tile_pool(name="psum_s", bufs=2, space="PSUM") as spool, \
                 tc.tile_pool(name="psum_o", bufs=3, space="PSUM") as opool, \
                 tc.tile_pool(name="psum_r2", bufs=1, space="PSUM") as rpool, \
                 tc.tile_pool(name="psum_pr", bufs=2, space="PSUM") as prpool, \
                 tc.tile_pool(name="pwork", bufs=3) as pw, \
                 tc.tile_pool(name="fwork", bufs=2) as fw, \
                 tc.tile_pool(name="obuf", bufs=3) as ob:
                for qtg in range(NQG):
                    o_tiles = {}
                    for h in range(2):
                        ops = opool.tile([P, QC], F32, tag="o", name="o_ps")
                        o_tiles[h] = ops
                        for qq in range(4):
                            qt = qtg * 4 + qq
                            kts = plan[qt]
                            groups = _chunk4(kts)
                            gi0 = 0
                            for g, grp in enumerate(groups):
                                w = len(grp) * P
                                sps = spool.tile([P, QC], F32, tag="s", name="s_ps")
                                for j, kt in enumerate(grp):
                                    nc.tensor.matmul(
                                        sps[:, j * P:(j + 1) * P],
                                        lhsT=qkvt[("k", h)][:, kt * P:(kt + 1) * P],
                                        rhs=qkvt[("q", h)][:, qt * P:(qt + 1) * P],
                                        start=True,
                                        stop=True,
                                    )
                                pb = pw.tile([P, QC], BF16, tag="p", name="p_sb")
                                nc.scalar.activation(
                                    pb[:, 0:w], sps[:, 0:w],
                                    mybir.ActivationFunctionType.Exp,
                                )
                                pm = pw.tile([P, QC], BF16, tag="pm", name="pm_sb")
                                co = (boff[qt] + gi0) * P
                                meng = nc.vector if h == 0 else nc.gpsimd
                                meng.tensor_tensor(
                                    out=pm[:, 0:w],
                                    in0=pb[:, 0:w],
                                    in1=ct_big[:, co:co + w],
                                    op=mybir.AluOpType.mult,
                                )
                                for j, kt in enumerate(grp):
                                    nc.tensor.matmul(
                                        ops[0:D + 1, qq * P:(qq + 1) * P],
                                        lhsT=v_sb[(h, kt)][:],
                                        rhs=pm[:, j * P:(j + 1) * P],
                                        start=(g == 0 and j == 0),
                                        stop=(g == len(groups) - 1
                                              and j == len(grp) - 1),
                                    )
                                gi0 += len(grp)
                        nc.vector.tensor_copy(
                            out=den_sb[h][0:1, qtg * QC:(qtg + 1) * QC],
                            in_=ops[D:D + 1, :],
                        )
                    # Phase 3 for this query group.
                    qs = slice(qtg * QC, (qtg + 1) * QC)
                    r2 = rpool.tile([P, QC], F32, tag="r2", name="r2_ps")
                    for h in range(2):
                        nc.tensor.matmul(
                            r2[:],
                            lhsT=sel_sb[h][0:1, :],
                            rhs=den_sb[h][0:1, qs],
                            start=(h == 0),
                            stop=(h == 1),
                        )
                    rr = fw.tile([P, QC], F32, tag="rr", name="rr_sb")
                    nc.vector.reciprocal_approx_fast(out=rr[:], in_=r2[:])
                    tmp = fw.tile([P, QC], F32, tag="tmp", name="tmp_sb")
                    for h in range(2):
                        hd = h * D
                        nc.vector.tensor_tensor(
                            out=tmp[hd:hd + D, :],
                            in0=o_tiles[h][0:D, :],
                            in1=rr[hd:hd + D, :],
                            op=mybir.AluOpType.mult,
                        )
                    nc.vector.tensor_tensor(
                        out=on_sb[:, qs], in0=tmp[:],
                        in1=bv_sb[:].to_broadcast([P, QC]),
                        op=mybir.AluOpType.add,
                    )
                    for qq in range(4):
                        qt = qtg * 4 + qq
                        pr = prpool.tile([P, DIM], F32, tag="pr", name="pr_ps")
                        nc.tensor.matmul(
                            pr[:],
                            lhsT=on_sb[:, qt * P:(qt + 1) * P],
                            rhs=wo_sb[:],
                            start=True,
                            stop=True,
                        )
                        o16 = ob.tile([P, DIM], FP16, tag="o16", name="o16_sb")
                        nc.scalar.copy(out=o16[:], in_=pr[:])
                        nc.sync.dma_start(
                            out=out_d[qt * P:(qt + 1) * P, :], in_=o16[:]
                        )

    nc.compile()
    return nc


def prepare(routes):
    routes = np.asarray(routes)
    key = routes.tobytes()
    if _CACHE.get("key") == key:
        return _CACHE["nc"], _CACHE["perm"], _CACHE["plan"], _CACHE["nblocks"]
    perm, plan, nblocks = plan_from_routes(routes)
    nc = build_nc(plan, nblocks)
    _CACHE.update(key=key, nc=nc, perm=perm, plan=plan, nblocks=nblocks)
    return nc, perm, plan, nblocks


# ----------------------------------------------------------------------
# Host-side data marshalling.

def make_in_maps(x, routes, w_qkv, b_qkv, w_out, perm, plan, nblocks):
    x = np.asarray(x, np.float32)
    routes = np.asarray(routes)
    w_qkv = np.asarray(w_qkv, np.float32)
    b_qkv = np.asarray(b_qkv, np.float32)
    w_out = np.asarray(w_out, np.float32)

    # count matrix in permuted space, packed per-block [keys, queries]
    C = np.zeros((S, S), np.float32)
    np.add.at(C, (np.arange(S)[:, None], routes), 1.0)
    Cp = C[np.ix_(perm, perm)]          # [q-pos, k-pos]
    blocks = []
    for qt in range(NT):
        for kt in plan[qt]:
            blocks.append(np.ascontiguousarray(
                Cp[qt * P:(qt + 1) * P, kt * P:(kt + 1) * P].T))
    ctp = np.concatenate(blocks, axis=1).astype(NPBF16)
    assert ctp.shape == (P, nblocks * P)

    def pack(a):
        # [n*128, X] -> [128, n*X]
        n = a.shape[0] // P
        return np.ascontiguousarray(
            a.reshape(n, P, a.shape[1]).transpose(1, 0, 2).reshape(P, -1))

    # x^T permuted, query-group-major: [128, (qc, c) blocks of 512]
    xts = []
    for b in range(B):
        xpT = np.ascontiguousarray(x[b][perm].T)        # [512, 2048]
        cols = []
        for qc in range(NQG):
            for c in range(NC4):
                cols.append(xpT[c * P:(c + 1) * P, qc * QC:(qc + 1) * QC])
        xts.append(np.concatenate(cols, axis=1).astype(NPBF16))

    in_maps = []
    for core in range(8):
        b = core // 4
        hp = core % 4
        col = hp * P
        wq = pack(w_qkv[:, col:col + P] * SCALE).astype(NPBF16)
        wk = pack(w_qkv[:, DIM + col:DIM + col + P]).astype(NPBF16)
        wv = pack(w_qkv[:, 2 * DIM + col:2 * DIM + col + P]).astype(NPBF16)
        bqk = np.stack([
            b_qkv[col:col + P] * SCALE,
            b_qkv[DIM + col:DIM + col + P],
        ]).astype(NPBF16)
        bv = b_qkv[2 * DIM + col:2 * DIM + col + P].astype(np.float32).reshape(P, 1)
        wo = np.ascontiguousarray(w_out[col:col + P, :]).astype(NPBF16)
        in_maps.append(dict(
            xt=xts[b], wq=wq, wk=wk, wv=wv, bqk=bqk, bv=bv,
            ct=ctp, wo=wo,
        ))
    return in_maps


def run(inputs, trace=False, trace_cores=None):
    nc, perm, plan, nblocks = prepare(inputs["routes"])
    in_maps = make_in_maps(
        inputs["x"], inputs["routes"], inputs["w_qkv"], inputs["b_qkv"],
        inputs["w_out"], perm, plan, nblocks,
    )
    res = run_bass_kernel_spmd(
        nc, in_maps, list(range(8)), trace=trace, trace_cores=trace_cores,
    )
    b_out = np.asarray(inputs["b_out"], np.float32)
    final = np.zeros((B, S, DIM), np.float32)
    for core in range(8):
        final[core // 4][perm] += np.asarray(
            res.results[core]["out"], np.float32)
    final += b_out[None, None, :]
    return final, res


def kernel(**inputs):
    final, _ = run(inputs, trace=False)
    return final


# revision 15
# speedup vs baseline: 1.0473x; 1.0473x over previous
"""CantorAttention Trainium2 kernel — banded block-sparse edition.

Problem (hardcoded): B=2, S=2048, DIM=512, H=8 heads, D=64, K=64 routes.
  qkv = x @ w_qkv + b_qkv ; per-head sparse attention over routes[q, :] ;
  out = attn_out @ w_out + b_out.

Strategy (8 cores): shard batch x head-pairs. Core i handles batch i//4 and
heads (2*(i%4), 2*(i%4)+1). Routes are shared across batch/heads.

The sparse attention is made BANDED by a host-computed permutation rho of
the sequence (barycenter seriation of the bipartite route graph — for the
Cantor routes this recovers coordinate order and every query's 64 routes
land in a <=128-wide window of permuted key space). Queries and keys are
both processed in rho order; only the nonzero 128x128 (key-tile x
query-tile) blocks of the routed score matrix are computed:

  P[k, q]  = C~[k, q] * exp(scale * (k_vec . k_vec))   (count mask, 0 off-route)
  AV_h     = [V_h | 1]^T @ P      -> rows 0..63 out, row 64 = denominator
  out_h    = AV_h / denom + bv_h  ;  partial = concat_h(out_h)^T @ w_out
Host gathers: final[b][perm] += partial_core ; final += b_out.

Exact softmax over the 64 routed scores for ANY routes input (block list
is derived from the actual routes; a bad permutation only costs speed).
All matmuls bf16 with fp32 PSUM accumulation; exp on ScalarE; mask-multiply
on VectorE; denominators broadcast across partitions via K=1 matmuls of
sel rows (no zeroed scratch needed); V is produced directly in [key, d]
layout (x^T blocks as stationary operand — no on-chip transposes);
input DMAs split across both HWDGE rings (sync + scalar) in critical-path
order; fp16 output stores.
"""

import numpy as np
import ml_dtypes

import concourse.bacc as bacc
import concourse.mybir as mybir
import concourse.tile as tile
from concourse.bass_utils import run_bass_kernel_spmd

BF16 = mybir.dt.bfloat16
F32 = mybir.dt.float32
FP16 = mybir.dt.float16
NPBF16 = ml_dtypes.bfloat16

B = 2
S = 2048
DIM = 512
H = 8
D = 64
KR = 64
SCALE = 0.125

P = 128
NT = S // P       # 16 tiles of 128 (queries and keys)
QC = 512          # query group width (psum bank)
NQG = S // QC     # 4 query groups
NC4 = DIM // P    # 4 contraction chunks
SGB = 4           # max blocks per (qt, h) score tile ([128, 512] psum)

_CACHE = {}


# ----------------------------------------------------------------------
# Host-side planning: permutation + block structure from routes alone.

def _block_cost(pos, routes):
    qt = pos[: routes.shape[0]] // P
    rp = pos[routes] // P
    return len(np.unique(qt[:, None] * NT + rp))


def _seriate(routes):
    """Find perm (rho-position -> original index) making the route matrix
    banded. Barycenter sweeps; keeps the best block count seen."""
    routes = np.asarray(routes, np.int64)
    n, k = routes.shape
    qidx = np.repeat(np.arange(n), k)
    kidx = routes.ravel()
    best_perm = np.arange(n)
    best_cost = _block_cost(np.arange(n), routes)
    for variant in ("q", "qk"):
        pos = np.arange(n, dtype=np.float64)
        stale = 0
        for _ in range(48):
            bq = pos[routes].mean(1)
            if variant == "qk":
                sums = np.zeros(n)
                cnts = np.zeros(n)
                np.add.at(sums, kidx, pos[qidx])
                np.add.at(cnts, kidx, 1)
                bk = np.where(cnts > 0, sums / np.maximum(cnts, 1.0), pos)
                b = 0.5 * (bq + bk)
            else:
                b = bq
            order = np.argsort(b, kind="stable")
            npos = np.empty(n)
            npos[order] = np.arange(n)
            pos = npos
            c = _block_cost(pos.astype(np.int64), routes)
            if c < best_cost:
                best_cost, best_perm, stale = c, order.copy(), 0
            else:
                stale += 1
                if stale >= 6:
                    break
    return best_perm, best_cost


def plan_from_routes(routes):
    """-> (perm, plan, nblocks); plan[qt] = tuple of key tiles needed."""
    routes = np.asarray(routes, np.int64)
    perm, _ = _seriate(routes)
    pos = np.empty(S, np.int64)
    pos[perm] = np.arange(S)
    plan = []
    for qt in range(NT):
        qs = perm[qt * P:(qt + 1) * P]
        kts = np.unique(pos[routes[qs]] // P)
        plan.append(tuple(int(x) for x in kts))
    nblocks = sum(len(x) for x in plan)
    return perm, tuple(plan), nblocks


def _chunk4(seq):
    return [seq[i:i + 4] for i in range(0, len(seq), 4)]


# ----------------------------------------------------------------------
# Device program.

def build_nc(plan, nblocks):
    nc = bacc.Bacc(
        "TRN2",
        target_bir_lowering=False,
        debug=False,
        num_devices=8,
    )

    xt_d = nc.dram_tensor("xt", [P, NC4 * S], BF16, kind="ExternalInput").ap()
    wq_d = nc.dram_tensor("wq", [P, NC4 * P], BF16, kind="ExternalInput").ap()
    wk_d = nc.dram_tensor("wk", [P, NC4 * P], BF16, kind="ExternalInput").ap()
    wv_d = nc.dram_tensor("wv", [P, NC4 * P], BF16, kind="ExternalInput").ap()
    # rows 0/1: q/k bias as [1, 128] rows (for the K=1 bias matmuls)
    bqk_d = nc.dram_tensor("bqk", [2, P], BF16, kind="ExternalInput").ap()
    bv_d = nc.dram_tensor("bv", [P, 1], F32, kind="ExternalInput").ap()
    ct_d = nc.dram_tensor("ct", [P, nblocks * P], BF16, kind="ExternalInput").ap()
    wo_d = nc.dram_tensor("wo", [P, DIM], BF16, kind="ExternalInput").ap()
    out_d = nc.dram_tensor("out", [S, DIM], FP16, kind="ExternalOutput").ap()

    # enum offset of first block of each query tile
    boff = np.cumsum([0] + [len(x) for x in plan])

    with tile.TileContext(nc) as tc:
        with tc.tile_pool(name="persist", bufs=1) as pp:
            # --- input DMAs, split across both HWDGE rings -------------
            xt_big = pp.tile([P, NC4 * S], BF16, tag="xtb", name="xt_big")
            w_sb = {}
            wts = {}
            for name, wd in (("q", wq_d), ("k", wk_d), ("v", wv_d)):
                wt = pp.tile([P, NC4 * P], BF16, tag=f"w{name}b", name=f"w{name}_big")
                wts[name] = wt
                for c in range(NC4):
                    w_sb[(name, c)] = wt[:, c * P:(c + 1) * P]
            bqk_sb = {}
            for bi in range(2):
                bqk_sb[bi] = pp.tile([P, P], BF16, tag=f"bqk{bi}", name=f"bqk{bi}_sb")
            bv_sb = pp.tile([P, 1], F32, tag="bv", name="bv_sb")
            ct_big = pp.tile([P, nblocks * P], BF16, tag="ctb", name="ct_big")
            wo_sb = pp.tile([P, DIM], BF16, tag="wo")

            nc.sync.dma_start(out=xt_big[:, 0:1024], in_=xt_d[:, 0:1024])
            nc.scalar.dma_start(out=wts["q"][:], in_=wq_d[:, :])
            nc.scalar.dma_start(out=wts["k"][:], in_=wk_d[:, :])
            nc.scalar.dma_start(out=bqk_sb[0][0:1, :], in_=bqk_d[0:1, :])
            nc.scalar.dma_start(out=bqk_sb[1][0:1, :], in_=bqk_d[1:2, :])
            nc.sync.dma_start(out=xt_big[:, 1024:2048], in_=xt_d[:, 1024:2048])
            nc.scalar.dma_start(out=wts["v"][:], in_=wv_d[:, :])
            lo, hi = 0, boff[4] * P
            nc.scalar.dma_start(out=ct_big[:, lo:hi], in_=ct_d[:, lo:hi])
            nc.sync.dma_start(out=xt_big[:, 2048:4096], in_=xt_d[:, 2048:4096])
            nc.sync.dma_start(out=xt_big[:, 4096:6144], in_=xt_d[:, 4096:6144])
            nc.sync.dma_start(out=xt_big[:, 6144:8192], in_=xt_d[:, 6144:8192])
            nc.scalar.dma_start(out=bv_sb[:], in_=bv_d[:, :])
            nc.scalar.dma_start(out=wo_sb[:], in_=wo_d[:, :])
            for g in range(1, NQG):
                lo, hi = boff[4 * g] * P, boff[4 * (g + 1)] * P
                nc.scalar.dma_start(out=ct_big[:, lo:hi], in_=ct_d[:, lo:hi])

            # --- small constants ---------------------------------------
            ones_sb = pp.tile([P, QC], BF16, tag="ones", name="ones_sb")
            nc.gpsimd.memset(ones_sb[0:1, :], 1.0)
            # sel rows for the K=1 denominator-broadcast matmuls: only row
            # 0 is ever read, so no full-tensor zeroing is needed anywhere.
            sel_sb = {}
            for h in range(2):
                t = pp.tile([P, P], BF16, tag=f"sel{h}", name=f"sel{h}")
                nc.gpsimd.memset(t[0:1, :], 0.0)
                nc.gpsimd.memset(t[0:1, h * D:(h + 1) * D], 1.0)
                sel_sb[h] = t

            # q^T/k^T per-head, rows 64-127 zero-padded (K=128 stationary
            # operands keep FWL enabled)
            qkvt = {}
            for ei, (name, h) in enumerate(
                    ((n, hh) for n in ("q", "k") for hh in range(2))):
                t = pp.tile([P, S], BF16, tag=f"{name}t{h}", name=f"{name}t{h}")
                nc.gpsimd.memset(t[D:P, :], 0.0)
                qkvt[(name, h)] = t
            # V tiles in [key, d] layout with a ones column at col 64
            v_sb = {}
            for h in range(2):
                for kt in range(NT):
                    t = pp.tile([P, D + 1], BF16, tag=f"v{h}_{kt}", name=f"v{h}_{kt}")
                    nc.gpsimd.memset(t[:, D:D + 1], 1.0)
                    v_sb[(h, kt)] = t

            den_sb = {}
            for h in range(2):
                den_sb[h] = pp.tile([P, S], BF16, tag=f"den{h}", name=f"den{h}")
            on_sb = pp.tile([P, S], BF16, tag="on")

            # Phase 1: per query group, K^T/Q^T (+bias) and V tiles.
            with tc.tile_pool(name="psum_pre", bufs=2, space="PSUM") as pre, \
                 tc.tile_pool(name="psum_v", bufs=4, space="PSUM") as vpl:
                for qc in range(NQG):
                    xq = xt_big[:, qc * 2048:(qc + 1) * 2048]
                    for bi, name in ((1, "k"), (0, "q")):
                        ps = pre.tile([P, QC], F32, tag="qkps", name="qkps")
                        for c in range(NC4):
                            nc.tensor.matmul(
                                ps[:],
                                lhsT=w_sb[(name, c)],
                                rhs=xq[:, c * QC:(c + 1) * QC],
                                start=(c == 0),
                                stop=False,
                            )
                        nc.tensor.matmul(
                            ps[:],
                            lhsT=bqk_sb[bi][0:1, :],
                            rhs=ones_sb[0:1, :],
                            start=False,
                            stop=True,
                        )
                        for h in range(2):
                            hd = h * D
                            nc.vector.tensor_copy(
                                out=qkvt[(name, h)][0:D, qc * QC:(qc + 1) * QC],
                                in_=ps[hd:hd + D, :],
                            )
                    for kk in range(4):
                        kt = qc * 4 + kk
                        vp = vpl.tile([P, P], F32, tag="vps", name="vps")
                        for c in range(NC4):
                            nc.tensor.matmul(
                                vp[:],
                                lhsT=xq[:, c * QC + kk * P: c * QC + (kk + 1) * P],
                                rhs=w_sb[("v", c)],
                                start=(c == 0),
                                stop=(c == NC4 - 1),
                            )
                        nc.vector.tensor_copy(out=v_sb[(0, kt)][:, 0:D], in_=vp[:, 0:D])
                        nc.vector.tensor_copy(out=v_sb[(1, kt)][:, 0:D], in_=vp[:, D:P])

            # Phase 2 + 3: banded masked attention, then normalize+project,
            # pipelined per 512-query group.
            with tc.tile_pool(name="psum_s", bufs=3, space="PSUM") as spool, \
                 tc.tile_pool(name="psum_o", bufs=3, space="PSUM") as opool, \
                 tc.tile_pool(name="psum_pr", bufs=2, space="PSUM") as prpool, \
                 tc.tile_pool(name="pwork", bufs=4) as pw, \
                 tc.tile_pool(name="fwork", bufs=2) as fw, \
                 tc.tile_pool(name="obuf", bufs=3) as ob:
                for qtg in range(NQG):
                    o_tiles = {}
                    for h in range(2):
                        ops = opool.tile([P, QC], F32, tag="o", name="o_ps")
                        o_tiles[h] = ops
                        for qq in range(4):
                            qt = qtg * 4 + qq
                            kts = plan[qt]
                            groups = _chunk4(kts)
                            gi0 = 0
                            for g, grp in enumerate(groups):
                                w = len(grp) * P
                                sps = spool.tile([P, QC], F32, tag="s", name="s_ps")
                                for j, kt in enumerate(grp):
                                    nc.tensor.matmul(
                                        sps[:, j * P:(j + 1) * P],
                                        lhsT=qkvt[("k", h)][:, kt * P:(kt + 1) * P],
                                        rhs=qkvt[("q", h)][:, qt * P:(qt + 1) * P],
                                        start=True,
                                        stop=True,
                                    )
                                pb = pw.tile([P, QC], BF16, tag="p", name="p_sb")
                                nc.scalar.activation(
                                    pb[:, 0:w], sps[:, 0:w],
                                    mybir.ActivationFunctionType.Exp,
                                )
                                pm = pw.tile([P, QC], BF16, tag="pm", name="pm_sb")
                                co = (boff[qt] + gi0) * P
                                nc.vector.tensor_tensor(
                                    out=pm[:, 0:w],
                                    in0=pb[:, 0:w],
                                    in1=ct_big[:, co:co + w],
                                    op=mybir.AluOpType.mult,
                                )
                                for j, kt in enumerate(grp):
                                    nc.tensor.matmul(
                                        ops[0:D + 1, qq * P:(qq + 1) * P],
                                        lhsT=v_sb[(h, kt)][:],
                                        rhs=pm[:, j * P:(j + 1) * P],
                                        start=(g == 0 and j == 0),
                                        stop=(g == len(groups) - 1
                                              and j == len(grp) - 1),
                                    )
                                gi0 += len(grp)
                        nc.vector.tensor_copy(
                            out=den_sb[h][0:1, qtg * QC:(qtg + 1) * QC],
                            in_=ops[D:D + 1, :],
                        )
                    # Phase 3 for this query group.
                    qs = slice(qtg * QC, (qtg + 1) * QC)
                    r2 = prpool.tile([P, QC], F32, tag="pr", name="r2_ps")
                    for h in range(2):
                        nc.tensor.matmul(
                            r2[:],
                            lhsT=sel_sb[h][0:1, :],
                            rhs=den_sb[h][0:1, qs],
                            start=(h == 0),
                            stop=(h == 1),
                        )
                    rr = fw.tile([P, QC], F32, tag="rr", name="rr_sb")
                    nc.vector.reciprocal_approx_fast(out=rr[:], in_=r2[:])
                    tmp = fw.tile([P, QC], F32, tag="tmp", name="tmp_sb")
                    for h in range(2):
                        hd = h * D
                        nc.vector.tensor_tensor(
                            out=tmp[hd:hd + D, :],
                            in0=o_tiles[h][0:D, :],
                            in1=rr[hd:hd + D, :],
                            op=mybir.AluOpType.mult,
                        )
                    nc.vector.tensor_tensor(
                        out=on_sb[:, qs], in0=tmp[:],
                        in1=bv_sb[:].to_broadcast([P, QC]),
                        op=mybir.AluOpType.add,
                    )
                    for qq in range(4):
                        qt = qtg * 4 + qq
                        pr = prpool.tile([P, DIM], F32, tag="pr", name="pr_ps")
                        nc.tensor.matmul(
                            pr[:],
                            lhsT=on_sb[:, qt * P:(qt + 1) * P],
                            rhs=wo_sb[:],
                            start=True,
                            stop=True,
                        )
                        o16 = ob.tile([P, DIM], FP16, tag="o16", name="o16_sb")
                        if qq % 2 == 0:
                            nc.vector.tensor_copy(out=o16[:], in_=pr[:])
                        else:
                            nc.scalar.copy(out=o16[:], in_=pr[:])
                        nc.sync.dma_start(
                            out=out_d[qt * P:(qt + 1) * P, :], in_=o16[:]
                        )

    nc.compile()
    return nc


def prepare(routes):
    routes = np.asarray(routes)
    key = routes.tobytes()
    if _CACHE.get("key") == key:
        return _CACHE["nc"], _CACHE["perm"], _CACHE["plan"], _CACHE["nblocks"]
    perm, plan, nblocks = plan_from_routes(routes)
    nc = build_nc(plan, nblocks)
    _CACHE.update(key=key, nc=nc, perm=perm, plan=plan, nblocks=nblocks)
    return nc, perm, plan, nblocks


# ----------------------------------------------------------------------
# Host-side data marshalling.

def make_in_maps(x, routes, w_qkv, b_qkv, w_out, perm, plan, nblocks):
    x = np.asarray(x, np.float32)
    routes = np.asarray(routes)
    w_qkv = np.asarray(w_qkv, np.float32)
    b_qkv = np.asarray(b_qkv, np.float32)
    w_out = np.asarray(w_out, np.float32)

    # count matrix in permuted space, packed per-block [keys, queries]
    C = np.zeros((S, S), np.float32)
    np.add.at(C, (np.arange(S)[:, None], routes), 1.0)
    Cp = C[np.ix_(perm, perm)]          # [q-pos, k-pos]
    blocks = []
    for qt in range(NT):
        for kt in plan[qt]:
            blocks.append(np.ascontiguousarray(
                Cp[qt * P:(qt + 1) * P, kt * P:(kt + 1) * P].T))
    ctp = np.concatenate(blocks, axis=1).astype(NPBF16)
    assert ctp.shape == (P, nblocks * P)

    def pack(a):
        # [n*128, X] -> [128, n*X]
        n = a.shape[0] // P
        return np.ascontiguousarray(
            a.reshape(n, P, a.shape[1]).transpose(1, 0, 2).reshape(P, -1))

    # x^T permuted, query-group-major: [128, (qc, c) blocks of 512]
    xts = []
    for b in range(B):
        xpT = np.ascontiguousarray(x[b][perm].T)        # [512, 2048]
        cols = []
        for qc in range(NQG):
            for c in range(NC4):
                cols.append(xpT[c * P:(c + 1) * P, qc * QC:(qc + 1) * QC])
        xts.append(np.concatenate(cols, axis=1).astype(NPBF16))

    in_maps = []
    for core in range(8):
        b = core // 4
        hp = core % 4
        col = hp * P
        wq = pack(w_qkv[:, col:col + P] * SCALE).astype(NPBF16)
        wk = pack(w_qkv[:, DIM + col:DIM + col + P]).astype(NPBF16)
        wv = pack(w_qkv[:, 2 * DIM + col:2 * DIM + col + P]).astype(NPBF16)
        bqk = np.stack([
            b_qkv[col:col + P] * SCALE,
            b_qkv[DIM + col:DIM + col + P],
        ]).astype(NPBF16)
        bv = b_qkv[2 * DIM + col:2 * DIM + col + P].astype(np.float32).reshape(P, 1)
        wo = np.ascontiguousarray(w_out[col:col + P, :]).astype(NPBF16)
        in_maps.append(dict(
            xt=xts[b], wq=wq, wk=wk, wv=wv, bqk=bqk, bv=bv,
            ct=ctp, wo=wo,
        ))
    return in_maps


def run(inputs, trace=False, trace_cores=None):
    nc, perm, plan, nblocks = prepare(inputs["routes"])
    in_maps = make_in_maps(
        inputs["x"], inputs["routes"], inputs["w_qkv"], inputs["b_qkv"],
        inputs["w_out"], perm, plan, nblocks,
    )
    res = run_bass_kernel_spmd(
        nc, in_maps, list(range(8)), trace=trace, trace_cores=trace_cores,
    )
    b_out = np.asarray(inputs["b_out"], np.float32)
    final = np.zeros((B, S, DIM), np.float32)
    for core in range(8):
        final[core // 4][perm] += np.asarray(
            res.results[core]["out"], np.float32)
    final += b_out[None, None, :]
    return final, res


def kernel(**inputs):
    final, _ = run(inputs, trace=False)
    return final


# revision 16
# speedup vs baseline: 1.1177x; 1.0672x over previous
"""CantorAttention Trainium2 kernel — banded block-sparse edition.

Problem (hardcoded): B=2, S=2048, DIM=512, H=8 heads, D=64, K=64 routes.
  qkv = x @ w_qkv + b_qkv ; per-head sparse attention over routes[q, :] ;
  out = attn_out @ w_out + b_out.

Strategy (8 cores): shard batch x head-pairs. Core i handles batch i//4 and
heads (2*(i%4), 2*(i%4)+1). Routes are shared across batch/heads.

The sparse attention is made BANDED by a host-computed permutation rho of
the sequence (barycenter seriation of the bipartite route graph — for the
Cantor routes this recovers coordinate order and every query's 64 routes
land in a <=128-wide window of permuted key space). Queries and keys are
both processed in rho order; only the nonzero 128x128 (key-tile x
query-tile) blocks of the routed score matrix are computed:

  P[k, q]  = C~[k, q] * exp(scale * (k_vec . k_vec))   (count mask, 0 off-route)
  AV_h     = [V_h | 1]^T @ P      -> rows 0..63 out, row 64 = denominator
  out_h    = AV_h / denom + bv_h  ;  partial = concat_h(out_h)^T @ w_out
Host gathers: final[b][perm] += partial_core ; final += b_out.

Exact softmax over the 64 routed scores for ANY routes input (block list
is derived from the actual routes; a bad permutation only costs speed).
All matmuls bf16 with fp32 PSUM accumulation; exp on ScalarE; mask-multiply
on VectorE; denominators broadcast across partitions via K=1 matmuls of
sel rows (no zeroed scratch needed); V is produced directly in [key, d]
layout (x^T blocks as stationary operand — no on-chip transposes);
input DMAs split across both HWDGE rings (sync + scalar) in critical-path
order; fp16 output stores.
"""

import numpy as np
import ml_dtypes

import concourse.bacc as bacc
import concourse.mybir as mybir
import concourse.tile as tile
from concourse.bass_utils import run_bass_kernel_spmd

BF16 = mybir.dt.bfloat16
F32 = mybir.dt.float32
FP16 = mybir.dt.float16
NPBF16 = ml_dtypes.bfloat16

B = 2
S = 2048
DIM = 512
H = 8
D = 64
KR = 64
SCALE = 0.125

P = 128
NT = S // P       # 16 tiles of 128 (queries and keys)
QC = 512          # query group width (psum bank)
NQG = S // QC     # 4 query groups
NC4 = DIM // P    # 4 contraction chunks
SGB = 4           # max blocks per (qt, h) score tile ([128, 512] psum)

_CACHE = {}


# ----------------------------------------------------------------------
# Host-side planning: permutation + block structure from routes alone.

def _block_cost(pos, routes):
    qt = pos[: routes.shape[0]] // P
    rp = pos[routes] // P
    return len(np.unique(qt[:, None] * NT + rp))


def _seriate(routes):
    """Find perm (rho-position -> original index) making the route matrix
    banded. Barycenter sweeps; keeps the best block count seen."""
    routes = np.asarray(routes, np.int64)
    n, k = routes.shape
    qidx = np.repeat(np.arange(n), k)
    kidx = routes.ravel()
    best_perm = np.arange(n)
    best_cost = _block_cost(np.arange(n), routes)
    for variant in ("q", "qk"):
        pos = np.arange(n, dtype=np.float64)
        stale = 0
        for _ in range(48):
            bq = pos[routes].mean(1)
            if variant == "qk":
                sums = np.zeros(n)
                cnts = np.zeros(n)
                np.add.at(sums, kidx, pos[qidx])
                np.add.at(cnts, kidx, 1)
                bk = np.where(cnts > 0, sums / np.maximum(cnts, 1.0), pos)
                b = 0.5 * (bq + bk)
            else:
                b = bq
            order = np.argsort(b, kind="stable")
            npos = np.empty(n)
            npos[order] = np.arange(n)
            pos = npos
            c = _block_cost(pos.astype(np.int64), routes)
            if c < best_cost:
                best_cost, best_perm, stale = c, order.copy(), 0
            else:
                stale += 1
                if stale >= 6:
                    break
    return best_perm, best_cost


def plan_from_routes(routes):
    """-> (perm, plan, nblocks); plan[qt] = tuple of key tiles needed."""
    routes = np.asarray(routes, np.int64)
    perm, _ = _seriate(routes)
    pos = np.empty(S, np.int64)
    pos[perm] = np.arange(S)
    plan = []
    for qt in range(NT):
        qs = perm[qt * P:(qt + 1) * P]
        kts = np.unique(pos[routes[qs]] // P)
        plan.append(tuple(int(x) for x in kts))
    nblocks = sum(len(x) for x in plan)
    return perm, tuple(plan), nblocks


def _chunk4(seq):
    return [seq[i:i + 4] for i in range(0, len(seq), 4)]


# ----------------------------------------------------------------------
# Device program.

def build_nc(plan, nblocks):
    nc = bacc.Bacc(
        "TRN2",
        target_bir_lowering=False,
        debug=False,
        num_devices=8,
    )

    xt_d = nc.dram_tensor("xt", [P, NC4 * S], BF16, kind="ExternalInput").ap()
    wq_d = nc.dram_tensor("wq", [P, NC4 * P], BF16, kind="ExternalInput").ap()
    wk_d = nc.dram_tensor("wk", [P, NC4 * P], BF16, kind="ExternalInput").ap()
    wv_d = nc.dram_tensor("wv", [P, NC4 * P], BF16, kind="ExternalInput").ap()
    bq_d = nc.dram_tensor("bq", [P, 1], F32, kind="ExternalInput").ap()
    bk_d = nc.dram_tensor("bk", [P, 1], F32, kind="ExternalInput").ap()
    bv_d = nc.dram_tensor("bv", [P, 1], F32, kind="ExternalInput").ap()
    ct_d = nc.dram_tensor("ct", [P, nblocks * P], BF16, kind="ExternalInput").ap()
    wo_d = nc.dram_tensor("wo", [P, DIM], BF16, kind="ExternalInput").ap()
    out_d = nc.dram_tensor("out", [S, DIM], FP16, kind="ExternalOutput").ap()

    # enum offset of first block of each query tile
    boff = np.cumsum([0] + [len(x) for x in plan])

    with tile.TileContext(nc) as tc:
        with tc.tile_pool(name="persist", bufs=1) as pp:
            # --- input DMAs, split across both HWDGE rings -------------
            xt_big = pp.tile([P, NC4 * S], BF16, tag="xtb", name="xt_big")
            w_sb = {}
            wts = {}
            for name, wd in (("q", wq_d), ("k", wk_d), ("v", wv_d)):
                wt = pp.tile([P, NC4 * P], BF16, tag=f"w{name}b", name=f"w{name}_big")
                wts[name] = wt
                for c in range(NC4):
                    w_sb[(name, c)] = wt[:, c * P:(c + 1) * P]
            b_sb = {}
            for name in ("q", "k"):
                b_sb[name] = pp.tile([P, 1], F32, tag=f"b{name}", name=f"b{name}_sb")
            bv_sb = pp.tile([P, 1], F32, tag="bv", name="bv_sb")
            ct_big = pp.tile([P, nblocks * P], BF16, tag="ctb", name="ct_big")
            wo_sb = pp.tile([P, DIM], BF16, tag="wo")

            nc.sync.dma_start(out=xt_big[:, 0:1024], in_=xt_d[:, 0:1024])
            nc.scalar.dma_start(out=wts["q"][:], in_=wq_d[:, :])
            nc.scalar.dma_start(out=wts["k"][:], in_=wk_d[:, :])
            nc.scalar.dma_start(out=b_sb["q"][:], in_=bq_d[:, :])
            nc.scalar.dma_start(out=b_sb["k"][:], in_=bk_d[:, :])
            nc.sync.dma_start(out=xt_big[:, 1024:2048], in_=xt_d[:, 1024:2048])
            nc.scalar.dma_start(out=wts["v"][:], in_=wv_d[:, :])
            lo, hi = 0, boff[4] * P
            nc.scalar.dma_start(out=ct_big[:, lo:hi], in_=ct_d[:, lo:hi])
            nc.sync.dma_start(out=xt_big[:, 2048:4096], in_=xt_d[:, 2048:4096])
            nc.sync.dma_start(out=xt_big[:, 4096:6144], in_=xt_d[:, 4096:6144])
            nc.sync.dma_start(out=xt_big[:, 6144:8192], in_=xt_d[:, 6144:8192])
            nc.scalar.dma_start(out=bv_sb[:], in_=bv_d[:, :])
            nc.scalar.dma_start(out=wo_sb[:], in_=wo_d[:, :])
            for g in range(1, NQG):
                lo, hi = boff[4 * g] * P, boff[4 * (g + 1)] * P
                nc.scalar.dma_start(out=ct_big[:, lo:hi], in_=ct_d[:, lo:hi])

            # --- small constants ---------------------------------------
            # sel rows for the K=1 denominator-broadcast matmuls: only row
            # 0 is ever read, so no full-tensor zeroing is needed anywhere.
            sel_sb = {}
            for h in range(2):
                t = pp.tile([P, P], BF16, tag=f"sel{h}", name=f"sel{h}")
                nc.gpsimd.memset(t[0:1, :], 0.0)
                nc.gpsimd.memset(t[0:1, h * D:(h + 1) * D], 1.0)
                sel_sb[h] = t

            # q^T/k^T per-head, rows 64-127 zero-padded (K=128 stationary
            # operands keep FWL enabled)
            qkvt = {}
            for ei, (name, h) in enumerate(
                    ((n, hh) for n in ("q", "k") for hh in range(2))):
                t = pp.tile([P, S], BF16, tag=f"{name}t{h}", name=f"{name}t{h}")
                if ei % 2 == 0:
                    nc.vector.memset(t[D:P, :], 0.0)
                else:
                    nc.gpsimd.memset(t[D:P, :], 0.0)
                qkvt[(name, h)] = t
            # V tiles in [key, d] layout with a ones column at col 64
            v_sb = {}
            for h in range(2):
                for kt in range(NT):
                    t = pp.tile([P, D + 1], BF16, tag=f"v{h}_{kt}", name=f"v{h}_{kt}")
                    nc.gpsimd.memset(t[:, D:D + 1], 1.0)
                    v_sb[(h, kt)] = t

            den_sb = {}
            for h in range(2):
                den_sb[h] = pp.tile([P, S], BF16, tag=f"den{h}", name=f"den{h}")
            on_sb = pp.tile([P, S], BF16, tag="on")

            # Phase 1: per query group, K^T/Q^T (+bias) and V tiles.
            with tc.tile_pool(name="psum_pre", bufs=2, space="PSUM") as pre, \
                 tc.tile_pool(name="psum_v", bufs=4, space="PSUM") as vpl:
                for qc in range(NQG):
                    xq = xt_big[:, qc * 2048:(qc + 1) * 2048]
                    for name in ("k", "q"):
                        ps = pre.tile([P, QC], F32, tag="qkps", name="qkps")
                        for c in range(NC4):
                            nc.tensor.matmul(
                                ps[:],
                                lhsT=w_sb[(name, c)],
                                rhs=xq[:, c * QC:(c + 1) * QC],
                                start=(c == 0),
                                stop=(c == NC4 - 1),
                            )
                        for h in range(2):
                            hd = h * D
                            nc.vector.tensor_tensor(
                                out=qkvt[(name, h)][0:D, qc * QC:(qc + 1) * QC],
                                in0=ps[hd:hd + D, :],
                                in1=b_sb[name][hd:hd + D, :].to_broadcast([D, QC]),
                                op=mybir.AluOpType.add,
                            )
                    for kk in range(4):
                        kt = qc * 4 + kk
                        vp = vpl.tile([P, P], F32, tag="vps", name="vps")
                        for c in range(NC4):
                            nc.tensor.matmul(
                                vp[:],
                                lhsT=xq[:, c * QC + kk * P: c * QC + (kk + 1) * P],
                                rhs=w_sb[("v", c)],
                                start=(c == 0),
                                stop=(c == NC4 - 1),
                            )
                        nc.vector.tensor_copy(out=v_sb[(0, kt)][:, 0:D], in_=vp[:, 0:D])
                        nc.vector.tensor_copy(out=v_sb[(1, kt)][:, 0:D], in_=vp[:, D:P])

            # Phase 2 + 3: banded masked attention, then normalize+project,
            # pipelined per 512-query group.
            with tc.tile_pool(name="psum_s", bufs=2, space="PSUM") as spool, \
                 tc.tile_pool(name="psum_o", bufs=3, space="PSUM") as opool, \
                 tc.tile_pool(name="psum_r2", bufs=1, space="PSUM") as rpool, \
                 tc.tile_pool(name="psum_pr", bufs=2, space="PSUM") as prpool, \
                 tc.tile_pool(name="pwork", bufs=3) as pw, \
                 tc.tile_pool(name="fwork", bufs=2) as fw, \
                 tc.tile_pool(name="obuf", bufs=3) as ob:
                for qtg in range(NQG):
                    o_tiles = {}
                    for h in range(2):
                        ops = opool.tile([P, QC], F32, tag="o", name="o_ps")
                        o_tiles[h] = ops
                        for qq in range(4):
                            qt = qtg * 4 + qq
                            kts = plan[qt]
                            groups = _chunk4(kts)
                            gi0 = 0
                            for g, grp in enumerate(groups):
                                w = len(grp) * P
                                sps = spool.tile([P, QC], F32, tag="s", name="s_ps")
                                for j, kt in enumerate(grp):
                                    nc.tensor.matmul(
                                        sps[:, j * P:(j + 1) * P],
                                        lhsT=qkvt[("k", h)][:, kt * P:(kt + 1) * P],
                                        rhs=qkvt[("q", h)][:, qt * P:(qt + 1) * P],
                                        start=True,
                                        stop=True,
                                    )
                                pb = pw.tile([P, QC], BF16, tag="p", name="p_sb")
                                nc.scalar.activation(
                                    pb[:, 0:w], sps[:, 0:w],
                                    mybir.ActivationFunctionType.Exp,
                                )
                                pm = pw.tile([P, QC], BF16, tag="pm", name="pm_sb")
                                co = (boff[qt] + gi0) * P
                                nc.vector.tensor_tensor(
                                    out=pm[:, 0:w],
                                    in0=pb[:, 0:w],
                                    in1=ct_big[:, co:co + w],
                                    op=mybir.AluOpType.mult,
                                )
                                for j, kt in enumerate(grp):
                                    nc.tensor.matmul(
                                        ops[0:D + 1, qq * P:(qq + 1) * P],
                                        lhsT=v_sb[(h, kt)][:],
                                        rhs=pm[:, j * P:(j + 1) * P],
                                        start=(g == 0 and j == 0),
                                        stop=(g == len(groups) - 1
                                              and j == len(grp) - 1),
                                    )
                                gi0 += len(grp)
                        nc.vector.tensor_copy(
                            out=den_sb[h][0:1, qtg * QC:(qtg + 1) * QC],
                            in_=ops[D:D + 1, :],
                        )
                    # Phase 3 for this query group.
                    qs = slice(qtg * QC, (qtg + 1) * QC)
                    r2 = rpool.tile([P, QC], F32, tag="r2", name="r2_ps")
                    for h in range(2):
                        nc.tensor.matmul(
                            r2[:],
                            lhsT=sel_sb[h][0:1, :],
                            rhs=den_sb[h][0:1, qs],
                            start=(h == 0),
                            stop=(h == 1),
                        )
                    rr = fw.tile([P, QC], F32, tag="rr", name="rr_sb")
                    nc.vector.reciprocal_approx_fast(out=rr[:], in_=r2[:])
                    tmp = fw.tile([P, QC], F32, tag="tmp", name="tmp_sb")
                    for h in range(2):
                        hd = h * D
                        nc.vector.tensor_tensor(
                            out=tmp[hd:hd + D, :],
                            in0=o_tiles[h][0:D, :],
                            in1=rr[hd:hd + D, :],
                            op=mybir.AluOpType.mult,
                        )
                    nc.vector.tensor_tensor(
                        out=on_sb[:, qs], in0=tmp[:],
                        in1=bv_sb[:].to_broadcast([P, QC]),
                        op=mybir.AluOpType.add,
                    )
                    for qq in range(4):
                        qt = qtg * 4 + qq
                        pr = prpool.tile([P, DIM], F32, tag="pr", name="pr_ps")
                        nc.tensor.matmul(
                            pr[:],
                            lhsT=on_sb[:, qt * P:(qt + 1) * P],
                            rhs=wo_sb[:],
                            start=True,
                            stop=True,
                        )
                        o16 = ob.tile([P, DIM], FP16, tag="o16", name="o16_sb")
                        if qq % 2 == 0:
                            nc.vector.tensor_copy(out=o16[:], in_=pr[:])
                        else:
                            nc.scalar.copy(out=o16[:], in_=pr[:])
                        nc.sync.dma_start(
                            out=out_d[qt * P:(qt + 1) * P, :], in_=o16[:]
                        )

    nc.compile()
    return nc


def prepare(routes):
    routes = np.asarray(routes)
    key = routes.tobytes()
    if _CACHE.get("key") == key:
        return _CACHE["nc"], _CACHE["perm"], _CACHE["plan"], _CACHE["nblocks"]
    perm, plan, nblocks = plan_from_routes(routes)
    nc = build_nc(plan, nblocks)
    _CACHE.update(key=key, nc=nc, perm=perm, plan=plan, nblocks=nblocks)
    return nc, perm, plan, nblocks


# ----------------------------------------------------------------------
# Host-side data marshalling.

def make_in_maps(x, routes, w_qkv, b_qkv, w_out, perm, plan, nblocks):
    x = np.asarray(x, np.float32)
    routes = np.asarray(routes)
    w_qkv = np.asarray(w_qkv, np.float32)
    b_qkv = np.asarray(b_qkv, np.float32)
    w_out = np.asarray(w_out, np.float32)

    # count matrix in permuted space, packed per-block [keys, queries]
    C = np.zeros((S, S), np.float32)
    np.add.at(C, (np.arange(S)[:, None], routes), 1.0)
    Cp = C[np.ix_(perm, perm)]          # [q-pos, k-pos]
    blocks = []
    for qt in range(NT):
        for kt in plan[qt]:
            blocks.append(np.ascontiguousarray(
                Cp[qt * P:(qt + 1) * P, kt * P:(kt + 1) * P].T))
    ctp = np.concatenate(blocks, axis=1).astype(NPBF16)
    assert ctp.shape == (P, nblocks * P)

    def pack(a):
        # [n*128, X] -> [128, n*X]
        n = a.shape[0] // P
        return np.ascontiguousarray(
            a.reshape(n, P, a.shape[1]).transpose(1, 0, 2).reshape(P, -1))

    # x^T permuted, query-group-major: [128, (qc, c) blocks of 512]
    xts = []
    for b in range(B):
        xpT = np.ascontiguousarray(x[b][perm].T)        # [512, 2048]
        cols = []
        for qc in range(NQG):
            for c in range(NC4):
                cols.append(xpT[c * P:(c + 1) * P, qc * QC:(qc + 1) * QC])
        xts.append(np.concatenate(cols, axis=1).astype(NPBF16))

    in_maps = []
    for core in range(8):
        b = core // 4
        hp = core % 4
        col = hp * P
        wq = pack(w_qkv[:, col:col + P] * SCALE).astype(NPBF16)
        wk = pack(w_qkv[:, DIM + col:DIM + col + P]).astype(NPBF16)
        wv = pack(w_qkv[:, 2 * DIM + col:2 * DIM + col + P]).astype(NPBF16)
        bq = (b_qkv[col:col + P] * SCALE).astype(np.float32).reshape(P, 1)
        bk = b_qkv[DIM + col:DIM + col + P].astype(np.float32).reshape(P, 1)
        bv = b_qkv[2 * DIM + col:2 * DIM + col + P].astype(np.float32).reshape(P, 1)
        wo = np.ascontiguousarray(w_out[col:col + P, :]).astype(NPBF16)
        in_maps.append(dict(
            xt=xts[b], wq=wq, wk=wk, wv=wv, bq=bq, bk=bk, bv=bv,
            ct=ctp, wo=wo,
        ))
    return in_maps


def run(inputs, trace=False, trace_cores=None):
    nc, perm, plan, nblocks = prepare(inputs["routes"])
    in_maps = make_in_maps(
        inputs["x"], inputs["routes"], inputs["w_qkv"], inputs["b_qkv"],
        inputs["w_out"], perm, plan, nblocks,
    )
    res = run_bass_kernel_spmd(
        nc, in_maps, list(range(8)), trace=trace, trace_cores=trace_cores,
    )
    b_out = np.asarray(inputs["b_out"], np.float32)
    final = np.zeros((B, S, DIM), np.float32)
    for core in range(8):
        final[core // 4][perm] += np.asarray(
            res.results[core]["out"], np.float32)
    final += b_out[None, None, :]
    return final, res


def kernel(**inputs):
    final, _ = run(inputs, trace=False)
    return final
